# revision 13
# baseline (speedup 1.0000x reference)
"""FFT-Conv2d (with buggy custom ifft2) — Trainium2 Bass kernel.

Math: the reference's custom ifft2 (missing final conj) reduces, after the
center crop, to
    out[b,o,r,c] = bias[o]                          for r<31 or c>33
    out[b,o,r,c] = bias[o] + g[b,o,96-r,32+c]       for 31<=r<=63, 0<=c<=33
where g = full linear conv of x (64x64) with w (3x3, true convolution),
summed over input channels.  So the device only computes the 33x34 region
g[p=33..65, q=32..65] (+bias); the host assembles the rest (bias broadcast).

Device mapping (per core, 2 batches): contraction K = 32 IC x 3 col-taps
(host-replicated with the column shift baked in) + 1 ones-row carrying the
bias => K=97.  3 accumulating matmuls (one per row-tap u) per chunk of 11
output rows (N=374), 3 chunks per batch.
"""

import os
import numpy as np
from contextlib import ExitStack

import concourse.bacc as bacc
import concourse.tile as tile
from concourse import mybir
from concourse.bass_utils import run_bass_kernel_spmd

N_CORES = 8
B, IC, H, W = 16, 32, 64, 64
OC = 64
BPC = B // N_CORES          # batches per core
KPART = 97                  # 3*32 taps + 1 ones row
PPAD = 128                  # padded partition count for DMA striping
TROWS, RCOLS = 35, 34       # x-patch tile rows (j=0..34 -> x rows 31..65), cols
RROWS = 33                  # g-region rows p = 33..65
CHUNK = 11                  # output rows per matmul chunk
NCHUNK = RROWS // CHUNK     # 3
NFREE = CHUNK * RCOLS       # 374

MM_DT_NAME = os.environ.get("KERNEL_MM_DT", "float16")
N_WARMUP = int(os.environ.get("KERNEL_N_WARMUP", "12"))

_cache = {}


def _mm_dt():
    return {
        "float32": mybir.dt.float32,
        "float32r": mybir.dt.float32r,
        "bfloat16": mybir.dt.bfloat16,
        "float16": mybir.dt.float16,
    }[MM_DT_NAME]


def _np_dt(mdt):
    return mybir.dt.np(mdt)


def _patch_tile_teardown():
    """Drop the second all-engine barrier in TileContext's teardown: the
    sem-range clear runs on Pool after the first barrier; other engines
    need not wait for it (the runtime joins all engine streams at NEFF
    end anyway)."""
    from concourse.vector_clock import ScopedClock

    def _drain_and_barrier(self, tick_clock, wait_clock):
        drain_inst = self.nc.sync.drain()
        wait_clock.add_sem_waits(
            drain_inst.ins, ScopedClock({None: tick_clock.global_clock})
        )
        self.nc.all_engine_barrier()
        popped = self.nc._tile_sem_poison_stack.pop()
        assert popped is self._sem_poison
        self.nc.clear_and_free_semaphores(list(self.sems.allocated().values()))

    tile.TileContext._drain_and_barrier = _drain_and_barrier


_patch_tile_teardown()


def _build(mm_dt):
    # Skip the barrier Bass.__init__ emits after its const-pool memsets —
    # this kernel never reads the const pool from another engine.
    orig_barrier = bacc.Bacc.all_engine_barrier
    bacc.Bacc.all_engine_barrier = lambda self, **kw: None
    try:
        nc = bacc.Bacc(
            "TRN2", target_bir_lowering=False, debug=False, num_devices=N_CORES
        )
    finally:
        bacc.Bacc.all_engine_barrier = orig_barrier
    xt_d = nc.dram_tensor(
        "xt", [BPC, PPAD, TROWS * RCOLS], mm_dt, kind="ExternalInput"
    ).ap()
    wt_d = nc.dram_tensor("wt", [PPAD, 3 * OC], mm_dt, kind="ExternalInput").ap()
    out_d = nc.dram_tensor(
        "out", [BPC, OC, RROWS, RCOLS], mybir.dt.float32, kind="ExternalOutput"
    ).ap()

    with tile.TileContext(nc) as tc, ExitStack() as ctx:
        wt_pool = ctx.enter_context(tc.tile_pool(name="wt", bufs=1))
        xt_pool = ctx.enter_context(tc.tile_pool(name="xt", bufs=2))
        ps_pool = ctx.enter_context(tc.tile_pool(name="ps", bufs=4, space="PSUM"))
        ob_pool = ctx.enter_context(tc.tile_pool(name="ob", bufs=4))
        warm_pool = ctx.enter_context(tc.tile_pool(name="warm", bufs=1))

        # Warm-up tile memset on ACT (scalar) — earliest free engine; the
        # PE warm-up matmuls flip the HAM clock gate (1.2 -> 2.4 GHz)
        # during the DMA wait so the real matmuls all run warm.
        warm = warm_pool.tile([PPAD, NFREE], mm_dt)
        nc.gpsimd.memset(warm[:, :], 0.0)

        # Inputs: both xt on the sync HWDGE ring — same-queue FIFO drains
        # xt0's packets before xt1's, so batch 0 compute starts at half
        # the transfer time; wt on SWDGE (gpsimd, striped). 128-partition
        # transfers stripe across all 16 SDMA engines.
        xts = []
        wt = wt_pool.tile([PPAD, 3 * OC], mm_dt)
        nc.gpsimd.dma_start(out=wt[:, :], in_=wt_d[:, :])
        for b in range(BPC):
            xt = xt_pool.tile([PPAD, TROWS * RCOLS], mm_dt)
            nc.sync.dma_start(out=xt[:, :], in_=xt_d[b, :, :])
            xts.append(xt)

        wps = ps_pool.tile([OC, NFREE], mybir.dt.float32, tag="warmps")
        for _ in range(N_WARMUP):
            nc.tensor.matmul(
                wps[:, :], warm[:, 0:OC], warm[:, :], start=True, stop=True
            )

        for b in range(BPC):
            xt = xts[b]
            for ch in range(NCHUNK):
                ps = ps_pool.tile([OC, NFREE], mybir.dt.float32)
                for u in range(3):
                    # chunk covers p = 33+11*ch .. +10; tile row j = p-u-31
                    j0 = 2 + CHUNK * ch - u
                    kk = KPART if u == 0 else KPART - 1
                    nc.tensor.matmul(
                        ps[:, :],
                        wt[0:kk, u * OC : (u + 1) * OC],
                        xt[0:kk, j0 * RCOLS : j0 * RCOLS + NFREE],
                        start=(u == 0),
                        stop=(u == 2),
                    )
                ob = ob_pool.tile([OC, NFREE], mybir.dt.float32)
                # alternate psum->sbuf copies between DVE and ACT so the
                # copy chain doesn't serialize behind the matmuls
                if ch % 2 == 0:
                    nc.vector.tensor_copy(ob[:, :], ps[:, :])
                else:
                    nc.scalar.copy(ob[:, :], ps[:, :])
                (nc.sync if ch % 2 == 0 else nc.scalar).dma_start(
                    out=out_d[b, :, ch * CHUNK : (ch + 1) * CHUNK, :],
                    in_=ob[:, :].rearrange("p (r c) -> p r c", c=RCOLS),
                )
    nc.compile()
    return nc


def _get_nc():
    key = MM_DT_NAME
    if key not in _cache:
        _cache[key] = _build(_mm_dt())
    return _cache[key]


LAST_RESULTS = None


def kernel(x, weight, bias):
    global LAST_RESULTS
    x = np.asarray(x, dtype=np.float32)
    weight = np.asarray(weight, dtype=np.float32)
    bias = np.asarray(bias, dtype=np.float32)
    np_dt = _np_dt(_mm_dt())

    # --- host prep: shard + im2col-lite (3 column-shifted replicas) ---
    xpad = np.zeros((B, IC, H + 2, W + 2), np.float32)
    xpad[:, :, :H, :W] = x
    XT = np.zeros((B, PPAD, TROWS, RCOLS), np.float32)
    for v in range(3):
        XT[:, v * 32 : (v + 1) * 32, :, :] = xpad[
            :, :, 31 : 31 + TROWS, 32 - v : 32 - v + RCOLS
        ]
    XT[:, 96] = 1.0
    XT = np.ascontiguousarray(XT.reshape(B, PPAD, TROWS * RCOLS)).astype(np_dt)

    WT = np.zeros((PPAD, 3 * OC), np.float32)
    # WT[v*32+i, u*64+oc] = weight[oc,i,u,v]
    WT[:96, :] = weight.transpose(3, 1, 2, 0).reshape(96, 3 * OC)
    WT[96, 0:OC] = bias
    WT = WT.astype(np_dt)

    nc = _get_nc()
    in_maps = [
        {"xt": XT[c * BPC : (c + 1) * BPC], "wt": WT} for c in range(N_CORES)
    ]
    res = run_bass_kernel_spmd(nc, in_maps, list(range(N_CORES)))
    LAST_RESULTS = res

    dev = np.stack([r["out"] for r in res.results])  # [8, BPC, OC, 33, 34]
    dev = dev.reshape(B, OC, RROWS, RCOLS)

    # --- host assembly: bias everywhere, conv region flipped in ---
    full = np.empty((B, OC, H, W), np.float32)
    full[:] = bias[None, :, None, None]
    full[:, :, 31:64, 0:34] = dev[:, :, ::-1, :]
    return full


# revision 17
# speedup vs baseline: 1.0670x; 1.0670x over previous
"""FFT-Conv2d (with buggy custom ifft2) — Trainium2 Bass kernel.

Math: the reference's custom ifft2 (missing final conj) reduces, after the
center crop, to
    out[b,o,r,c] = bias[o]                          for r<31 or c>33
    out[b,o,r,c] = bias[o] + g[b,o,96-r,32+c]       for 31<=r<=63, 0<=c<=33
where g = full linear conv of x (64x64) with w (3x3, true convolution),
summed over input channels.  So the device only computes the 33x34 region
g[p=33..65, q=32..65] (+bias); the host assembles the rest (bias broadcast).

Device mapping (per core, 2 batches): contraction K = 32 IC x 3 col-taps
(host-replicated with the column shift baked in) + 1 ones-row carrying the
bias => K=97.  3 accumulating matmuls (one per row-tap u) per chunk of 11
output rows (N=374), 3 chunks per batch.
"""

import os
import numpy as np
from contextlib import ExitStack

import concourse.bacc as bacc
import concourse.tile as tile
from concourse import mybir
from concourse.bass_utils import run_bass_kernel_spmd

N_CORES = 8
B, IC, H, W = 16, 32, 64, 64
OC = 64
BPC = B // N_CORES          # batches per core
KPART = 97                  # 3*32 taps + 1 ones row
PPAD = 128                  # padded partition count for DMA striping
TROWS, RCOLS = 35, 34       # x-patch tile rows (j=0..34 -> x rows 31..65), cols
RROWS = 33                  # g-region rows p = 33..65
CHUNK = 11                  # output rows per matmul chunk
NCHUNK = RROWS // CHUNK     # 3
NFREE = CHUNK * RCOLS       # 374

MM_DT_NAME = os.environ.get("KERNEL_MM_DT", "float16")
N_WARMUP = int(os.environ.get("KERNEL_N_WARMUP", "12"))

_cache = {}


def _mm_dt():
    return {
        "float32": mybir.dt.float32,
        "float32r": mybir.dt.float32r,
        "bfloat16": mybir.dt.bfloat16,
        "float16": mybir.dt.float16,
    }[MM_DT_NAME]


def _np_dt(mdt):
    return mybir.dt.np(mdt)


def _patch_tile_teardown():
    """Drop the second all-engine barrier in TileContext's teardown: the
    sem-range clear runs on Pool after the first barrier; other engines
    need not wait for it (the runtime joins all engine streams at NEFF
    end anyway)."""
    from concourse.vector_clock import ScopedClock

    def _drain_and_barrier(self, tick_clock, wait_clock):
        drain_inst = self.nc.sync.drain()
        wait_clock.add_sem_waits(
            drain_inst.ins, ScopedClock({None: tick_clock.global_clock})
        )
        self.nc.all_engine_barrier()
        popped = self.nc._tile_sem_poison_stack.pop()
        assert popped is self._sem_poison
        self.nc.clear_and_free_semaphores(list(self.sems.allocated().values()))

    tile.TileContext._drain_and_barrier = _drain_and_barrier


_patch_tile_teardown()


def _build(mm_dt):
    # Skip the barrier Bass.__init__ emits after its const-pool memsets —
    # this kernel never reads the const pool from another engine.
    orig_barrier = bacc.Bacc.all_engine_barrier
    bacc.Bacc.all_engine_barrier = lambda self, **kw: None
    try:
        nc = bacc.Bacc(
            "TRN2", target_bir_lowering=False, debug=False, num_devices=N_CORES
        )
    finally:
        bacc.Bacc.all_engine_barrier = orig_barrier
    xt_d = nc.dram_tensor(
        "xt", [BPC, PPAD, TROWS * RCOLS], mm_dt, kind="ExternalInput"
    ).ap()
    wt_d = nc.dram_tensor("wt", [PPAD, 3 * OC], mm_dt, kind="ExternalInput").ap()
    out_d = nc.dram_tensor(
        "out", [BPC, OC, RROWS, RCOLS], mybir.dt.float32, kind="ExternalOutput"
    ).ap()

    with tile.TileContext(nc) as tc, ExitStack() as ctx:
        wt_pool = ctx.enter_context(tc.tile_pool(name="wt", bufs=1))
        xt_pool = ctx.enter_context(tc.tile_pool(name="xt", bufs=2))
        ps_pool = ctx.enter_context(tc.tile_pool(name="ps", bufs=6, space="PSUM"))
        ob_pool = ctx.enter_context(tc.tile_pool(name="ob", bufs=6))
        warm_pool = ctx.enter_context(tc.tile_pool(name="warm", bufs=1))

        # Warm-up tile memset on ACT (scalar) — earliest free engine; the
        # PE warm-up matmuls flip the HAM clock gate (1.2 -> 2.4 GHz)
        # during the DMA wait so the real matmuls all run warm.
        # raw (non-pool) SBUF buffer: contents are garbage, which is fine
        # for warm-up matmuls, and needs no producing write to schedule
        warm = nc.alloc_sbuf_tensor("warmbuf", [PPAD, NFREE], mm_dt).ap()
        if os.environ.get("KERNEL_WARM_MEMSET", "0") == "1":
            nc.gpsimd.memset(warm[:, :], 0.0)

        # Inputs: both xt on the sync HWDGE ring — same-queue FIFO drains
        # xt0's packets before xt1's, so batch 0 compute starts at half
        # the transfer time; wt on SWDGE (gpsimd, striped). 128-partition
        # transfers stripe across all 16 SDMA engines.
        xts = []
        wt = wt_pool.tile([PPAD, 3 * OC], mm_dt)
        nc.gpsimd.dma_start(out=wt[:, :], in_=wt_d[:, :])
        for b in range(BPC):
            xt = xt_pool.tile([PPAD, TROWS * RCOLS], mm_dt)
            nc.sync.dma_start(out=xt[:, :], in_=xt_d[b, :, :])
            xts.append(xt)

        wps = ps_pool.tile([OC, NFREE], mybir.dt.float32, tag="warmps", bufs=1)
        for _ in range(N_WARMUP):
            nc.tensor.matmul(
                wps[:, :], warm[:, 0:OC], warm[:, :], start=True, stop=True
            )

        for b in range(BPC):
            xt = xts[b]
            for ch in range(NCHUNK):
                ps = ps_pool.tile([OC, NFREE], mybir.dt.float32)
                for u in range(3):
                    # chunk covers p = 33+11*ch .. +10; tile row j = p-u-31
                    j0 = 2 + CHUNK * ch - u
                    kk = KPART if u == 0 else KPART - 1
                    nc.tensor.matmul(
                        ps[:, :],
                        wt[0:kk, u * OC : (u + 1) * OC],
                        xt[0:kk, j0 * RCOLS : j0 * RCOLS + NFREE],
                        start=(u == 0),
                        stop=(u == 2),
                    )
                ob = ob_pool.tile([OC, NFREE], mybir.dt.float32)
                # alternate psum->sbuf copies between DVE and ACT so the
                # copy chain doesn't serialize behind the matmuls
                if ch % 2 == 0:
                    nc.vector.tensor_copy(ob[:, :], ps[:, :])
                else:
                    nc.scalar.copy(ob[:, :], ps[:, :])
                (nc.sync if ch % 2 == 0 else nc.scalar).dma_start(
                    out=out_d[b, :, ch * CHUNK : (ch + 1) * CHUNK, :],
                    in_=ob[:, :].rearrange("p (r c) -> p r c", c=RCOLS),
                )
    nc.compile()
    return nc


def _get_nc():
    key = MM_DT_NAME
    if key not in _cache:
        _cache[key] = _build(_mm_dt())
    return _cache[key]


LAST_RESULTS = None


def kernel(x, weight, bias):
    global LAST_RESULTS
    x = np.asarray(x, dtype=np.float32)
    weight = np.asarray(weight, dtype=np.float32)
    bias = np.asarray(bias, dtype=np.float32)
    np_dt = _np_dt(_mm_dt())

    # --- host prep: shard + im2col-lite (3 column-shifted replicas) ---
    xpad = np.zeros((B, IC, H + 2, W + 2), np.float32)
    xpad[:, :, :H, :W] = x
    XT = np.zeros((B, PPAD, TROWS, RCOLS), np.float32)
    for v in range(3):
        XT[:, v * 32 : (v + 1) * 32, :, :] = xpad[
            :, :, 31 : 31 + TROWS, 32 - v : 32 - v + RCOLS
        ]
    XT[:, 96] = 1.0
    XT = np.ascontiguousarray(XT.reshape(B, PPAD, TROWS * RCOLS)).astype(np_dt)

    WT = np.zeros((PPAD, 3 * OC), np.float32)
    # WT[v*32+i, u*64+oc] = weight[oc,i,u,v]
    WT[:96, :] = weight.transpose(3, 1, 2, 0).reshape(96, 3 * OC)
    WT[96, 0:OC] = bias
    WT = WT.astype(np_dt)

    nc = _get_nc()
    in_maps = [
        {"xt": XT[c * BPC : (c + 1) * BPC], "wt": WT} for c in range(N_CORES)
    ]
    res = run_bass_kernel_spmd(nc, in_maps, list(range(N_CORES)))
    LAST_RESULTS = res

    dev = np.stack([r["out"] for r in res.results])  # [8, BPC, OC, 33, 34]
    dev = dev.reshape(B, OC, RROWS, RCOLS)

    # --- host assembly: bias everywhere, conv region flipped in ---
    full = np.empty((B, OC, H, W), np.float32)
    full[:] = bias[None, :, None, None]
    full[:, :, 31:64, 0:34] = dev[:, :, ::-1, :]
    return full
